# revision 17
# baseline (speedup 1.0000x reference)
"""Bipolar morphological conv2d kernel for Trainium2 (8 NeuronCores).

Math: reference computes, per output position and out-channel c,
    y = m(lp1,K1) - m(lp1,K2) - m(lp2,K1) + m(lp2,K2) + bias
with m(logp, k)[c] = exp(max_p(logp_p + k_pc)), lp1 = log(max(patch, .1)),
lp2 = log(max(-patch, .1)).

Since exp is monotone, m(lp1,K)[c] = max(U_c, max_p(x_p*K_pc)) and
m(lp2,K)[c] = max(U_c, -min_p(x_p*K_pc)) with K = exp(k) > 0 and
U_c = .1*max_p K_pc (the clamp folds into a per-channel constant).  So the
whole op needs ONE product set per kernel, max- AND min-reduced over taps:
    y = (mA1 - mA2) + (aMin1 - aMin2) + bias
with mA_k = max(U_k, max_p x_p*K_k), aMin_k = min(-U_k, min_p x_p*K_k).

Device strategy (data-parallel, one batch image per core):
  - partitions = 128 = [64 out-channels of K1 | 64 out-channels of K2]
  - free dim = 900 output positions as [30 rows, 30 cols] windows (row
    stride 32) into a per-ci broadcast row; host pre-replicates the rows
    across partitions in DRAM (fp16, even+odd parity copies so every tap
    window is 4B aligned).  The per-(tap,ci) kernel scalars and the U
    clamps ride along as trailing columns of the same tile, so one DMA
    per ci delivers everything and nothing else gates the first product.
  - products on the Activation engine (Copy with per-partition scale;
    Pool rejects TensorScalarPtr/TensorTensor at codegen), grouped 4 taps
    per buffer; the seed group and a small sliver run on DVE to balance.
  - folds on DVE: per group one tensor_tensor max + one min over
    [128, 4*900] fp16 (2x_1p mode). U clamp applied once post-merge.
  - tail, pipelined in two PSUM-bank-aligned column halves: merge the
    sub-accumulators, clamp at +-U, then accumulating PE matmuls with
    lhsT=[I;-I] turn the accs into (col_K1 - col_K2) sums with channels
    on PARTITIONS ([64, *] PSUM — 64 DMA descriptor lines instead of
    900), Activation adds the bias while staging PSUM->SBUF, DMA out
    Y [64, 900] (host transposes back).
"""

import os
from contextlib import ExitStack

import numpy as np

import concourse.bass as bass
import concourse.mybir as mybir
from concourse import bacc
import concourse.tile as tile
from concourse.bass_utils import run_bass_kernel_spmd

N_CORES = 8
H = W = C = 32
COUT = 64
HO = WO = 30
NPIX = H * W          # 1024
FD = HO * WO          # 900 output positions, accessed as [30, 30] windows
ROWL = 1026           # even-parity row length (1024 pixels + 2 pad)
SCOL = 2 * ROWL       # fp32 per-tap scalars, packed as fp16 slot pairs
UCOL = SCOL + 18      # [U, -U] fp32 columns (2 slots each)
XLEN = UCOL + 4       # row length in fp16 slots (4B aligned)
P = 288               # 3*3*32 patch size
G = 4                 # taps per product buffer / fold group

F32 = mybir.dt.float32
F16 = mybir.dt.float16
_cache: dict = {}
last_results = None


def _ensure_axon_ntff_hook():
    """The trimmed agent image lacks antenv.axon_hooks; recreate it so
    run_bass_kernel_spmd(trace=True) can capture NTFF profiles. No-op on
    failure (tracing then just degrades)."""
    import sys
    import types

    try:
        import antenv.axon_hooks  # noqa: F401
        return
    except ImportError:
        pass
    try:
        mod = types.ModuleType("antenv.axon_hooks")
        holder = [None]
        mod.set_axon_ntff_profile_hook = lambda h: holder.__setitem__(0, h)
        mod.get_axon_ntff_profile_hook = lambda: holder[0]
        sys.modules["antenv.axon_hooks"] = mod
        from trn_agent_boot.trn_boot import _ntff_profile_via_ctypes

        so = "/opt/axon/libaxon_pjrt.so"
        if os.path.exists(so):
            holder[0] = _ntff_profile_via_ctypes(so)
    except Exception:
        pass


def _build_module():
    nc = bacc.Bacc()
    Alu = mybir.AluOpType

    XB = nc.dram_tensor("XB", [C * 128, XLEN], F16, kind="ExternalInput")
    M1 = nc.dram_tensor("M1", [128, COUT], F16, kind="ExternalInput")
    BCc = nc.dram_tensor("BCc", [COUT, 1], F32, kind="ExternalInput")
    Y = nc.dram_tensor("Y", [COUT, FD], F32, kind="ExternalOutput")

    with tile.TileContext(nc) as tc, ExitStack() as ctx:
        const = ctx.enter_context(tc.tile_pool(name="const", bufs=1))
        xbp = ctx.enter_context(tc.tile_pool(name="xbp", bufs=6))
        pbp = ctx.enter_context(tc.tile_pool(name="pbp", bufs=8))
        accp = ctx.enter_context(tc.tile_pool(name="accp", bufs=1))
        tps = ctx.enter_context(tc.tile_pool(name="tps", bufs=1, space="PSUM"))
        tsb = ctx.enter_context(tc.tile_pool(name="tsb", bufs=1))

        # xb[0] gates the first products: issue it before everything else.
        xb0 = xbp.tile([128, XLEN], F16, tag="xb")
        nc.sync.dma_start(out=xb0[:, :], in_=XB[0:128, :])
        M1_sb = const.tile([128, COUT], F16)
        nc.gpsimd.dma_start(out=M1_sb[:, :], in_=M1[:, :])
        BC_sb = const.tile([COUT, 1], F32)
        nc.gpsimd.dma_start(out=BC_sb[:, :], in_=BCc[:, :])

        accMax = accp.tile([128, G * FD], F16)
        accMin = accp.tile([128, G * FD], F16)

        pb = None
        xbf = xb0  # tile holding the (identical) U columns, kept live
        for ci in range(C):
            if ci == 0:
                xb_sb = xb0
            else:
                xb_sb = xbp.tile([128, XLEN], F16, tag="xb")
                nc.sync.dma_start(
                    out=xb_sb[:, :], in_=XB[ci * 128 : (ci + 1) * 128, :])
                if ci == C - 1:
                    xbf = xb_sb
            for t in range(9):
                i, j = divmod(t, 3)
                base = (ROWL + i * W) if j == 1 else (i * W + j)
                win = xb_sb[:, base : base + HO * W].rearrange(
                    "q (a b) -> q a b", b=W)[:, :, :WO]
                k = ci * 9 + t
                sc = xb_sb[:, SCOL + 2 * t : SCOL + 2 * t + 2].bitcast(F32)
                g, slot = divmod(k, G)
                if g == 0:
                    # first group seeds accMax directly; accMin is copied
                    # from it once (below) instead of duplicating products
                    dst = accMax
                elif slot == 0:
                    pb = pbp.tile([128, G * FD], F16, tag="pb")
                    dst = pb
                else:
                    dst = pb
                out_view = dst[:, slot * FD : (slot + 1) * FD].rearrange(
                    "q (a b) -> q a b", a=HO)
                if g == 0 or (k % 40 == 39 and g < 71):
                    # seed group + a sliver of products run on DVE: it is
                    # idle during ramp-up and slightly under Act's load
                    nc.vector.tensor_scalar(
                        out=out_view, in0=win, scalar1=sc, scalar2=None,
                        op0=Alu.mult)
                else:
                    nc.scalar.mul(out=out_view, in_=win, mul=sc)
                if g == 0 and slot == G - 1:
                    nc.vector.tensor_scalar(
                        out=accMin[:, :], in0=accMax[:, :], scalar1=0.0,
                        scalar2=None, op0=Alu.add)
                if 0 < g < 71 and slot == G - 1:
                    nc.vector.tensor_tensor(
                        accMax[:, :], pb[:, :], accMax[:, :], Alu.max)
                    nc.vector.tensor_tensor(
                        accMin[:, :], pb[:, :], accMin[:, :], Alu.min)

        # Tail. The accumulator merge tree (groups 0..70) runs EARLY --
        # hidden behind the Act products of the final group -- and the last
        # group's buffer is folded by a pairwise tree per column half, so
        # only ~5us of DVE work remains after the last product.
        tmpx = accp.tile([128, 2 * FD], F16)
        tmpn = accp.tile([128, 2 * FD], F16)
        M0x = accp.tile([128, FD], F16)
        M0n = accp.tile([128, FD], F16)
        nc.vector.tensor_tensor(
            tmpx[:, :], accMax[:, : 2 * FD], accMax[:, 2 * FD :], Alu.max)
        nc.vector.tensor_tensor(
            M0x[:, :], tmpx[:, :FD], tmpx[:, FD:], Alu.max)
        nc.vector.tensor_tensor(
            tmpn[:, :], accMin[:, : 2 * FD], accMin[:, 2 * FD :], Alu.min)
        nc.vector.tensor_tensor(
            M0n[:, :], tmpn[:, :FD], tmpn[:, FD:], Alu.min)

        Mx = accp.tile([128, FD], F16)
        Mn = accp.tile([128, FD], F16)
        t71x = accp.tile([128, 2 * FD], F16)
        t71n = accp.tile([128, 2 * FD], F16)
        pt = tps.tile([128, FD], F32)
        y32 = tsb.tile([COUT, FD], F32)
        pb4f = pb[:, :].rearrange("q (u f) -> q u f", f=FD)
        t71x2 = t71x[:, :].rearrange("q (u f) -> q u f", f=FD)
        t71n2 = t71n[:, :].rearrange("q (u f) -> q u f", f=FD)
        for s, e in ((0, 512), (512, FD)):
            nc.vector.tensor_tensor(
                t71x2[:, :, s:e], pb4f[:, 0:2, s:e], pb4f[:, 2:4, s:e], Alu.max)
            nc.vector.tensor_tensor(
                Mx[:, s:e], t71x[:, s:e], t71x[:, FD + s : FD + e], Alu.max)
            nc.vector.tensor_tensor(
                Mx[:, s:e], Mx[:, s:e], M0x[:, s:e], Alu.max)
            nc.vector.tensor_scalar(
                out=Mx[:, s:e], in0=Mx[:, s:e],
                scalar1=xbf[:, UCOL : UCOL + 2].bitcast(F32), scalar2=None,
                op0=Alu.max)
            nc.vector.tensor_tensor(
                t71n2[:, :, s:e], pb4f[:, 0:2, s:e], pb4f[:, 2:4, s:e], Alu.min)
            nc.vector.tensor_tensor(
                Mn[:, s:e], t71n[:, s:e], t71n[:, FD + s : FD + e], Alu.min)
            nc.vector.tensor_tensor(
                Mn[:, s:e], Mn[:, s:e], M0n[:, s:e], Alu.min)
            nc.vector.tensor_scalar(
                out=Mn[:, s:e], in0=Mn[:, s:e],
                scalar1=xbf[:, UCOL + 2 : UCOL + 4].bitcast(F32), scalar2=None,
                op0=Alu.min)
            nc.tensor.matmul(pt[:COUT, s:e], lhsT=M1_sb[:, :], rhs=Mx[:, s:e],
                             start=True, stop=False)
            nc.tensor.matmul(pt[:COUT, s:e], lhsT=M1_sb[:, :], rhs=Mn[:, s:e],
                             start=False, stop=True)
            nc.scalar.activation(
                out=y32[:, s:e], in_=pt[:COUT, s:e],
                func=mybir.ActivationFunctionType.Identity,
                bias=BC_sb[:, 0:1], scale=1.0)
            nc.sync.dma_start(out=Y[:, s:e], in_=y32[:, s:e])
    nc.finalize()
    return nc


def _host_prep(x, k1, k2, bias):
    x = np.ascontiguousarray(np.asarray(x, dtype=np.float32))
    K1 = np.exp(np.asarray(k1, np.float32).reshape(3, 3, C, COUT))
    K2 = np.exp(np.asarray(k2, np.float32).reshape(3, 3, C, COUT))
    # S[q, ci, t=i*3+j]: q<64 -> K1[i,j,ci,q];  q>=64 -> K2[i,j,ci,q-64]
    S1 = K1.transpose(3, 2, 0, 1).reshape(COUT, C, 9)
    S2 = K2.transpose(3, 2, 0, 1).reshape(COUT, C, 9)
    S = np.concatenate([S1, S2], axis=0).astype(np.float32)   # [128, C, 9]
    U1 = 0.1 * K1.reshape(9 * C, COUT).max(axis=0)
    U2_ = 0.1 * K2.reshape(9 * C, COUT).max(axis=0)
    U = np.concatenate([U1, U2_]).astype(np.float32)          # [128]
    M1 = np.vstack([np.eye(COUT, dtype=np.float16), -np.eye(COUT, dtype=np.float16)])
    BCc = np.asarray(bias, np.float32).reshape(COUT, 1)
    shared = dict(M1=np.ascontiguousarray(M1), BCc=np.ascontiguousarray(BCc))
    in_maps = []
    for n in range(N_CORES):
        rows = np.zeros((C, XLEN), np.float16)
        xr = x[n].reshape(NPIX, C).T.astype(np.float16)       # [C, 1024]
        rows[:, :NPIX] = xr
        rows[:, ROWL : ROWL + NPIX - 1] = xr[:, 1:]
        xb = np.broadcast_to(rows[:, None, :], (C, 128, XLEN)).copy()
        xb[:, :, SCOL:UCOL].view(np.float32)[:] = S.transpose(1, 0, 2)
        xb[:, :, UCOL : UCOL + 2].view(np.float32)[:, :, 0] = U[None, :]
        xb[:, :, UCOL + 2 : UCOL + 4].view(np.float32)[:, :, 0] = -U[None, :]
        in_maps.append({"XB": xb.reshape(C * 128, XLEN), **shared})
    return in_maps


def kernel(x, k1, k2, bias):
    global last_results
    if "nc" not in _cache:
        _cache["nc"] = _build_module()
    nc = _cache["nc"]
    in_maps = _host_prep(x, k1, k2, bias)
    trace = bool(int(os.environ.get("KTRACE", "0")))
    if trace:
        _ensure_axon_ntff_hook()
    res = run_bass_kernel_spmd(
        nc, in_maps, core_ids=list(range(N_CORES)), trace=trace,
    )
    last_results = res
    y = np.stack([r["Y"].reshape(COUT, HO, WO).transpose(1, 2, 0)
                  for r in res.results], axis=0)
    return np.ascontiguousarray(y, np.float32)


# revision 18
# speedup vs baseline: 1.0189x; 1.0189x over previous
"""Bipolar morphological conv2d kernel for Trainium2 (8 NeuronCores).

Math: reference computes, per output position and out-channel c,
    y = m(lp1,K1) - m(lp1,K2) - m(lp2,K1) + m(lp2,K2) + bias
with m(logp, k)[c] = exp(max_p(logp_p + k_pc)), lp1 = log(max(patch, .1)),
lp2 = log(max(-patch, .1)).

Since exp is monotone, m(lp1,K)[c] = max(U_c, max_p(x_p*K_pc)) and
m(lp2,K)[c] = max(U_c, -min_p(x_p*K_pc)) with K = exp(k) > 0 and
U_c = .1*max_p K_pc (the clamp folds into a per-channel constant).  So the
whole op needs ONE product set per kernel, max- AND min-reduced over taps:
    y = (mA1 - mA2) + (aMin1 - aMin2) + bias
with mA_k = max(U_k, max_p x_p*K_k), aMin_k = min(-U_k, min_p x_p*K_k).

Device strategy (data-parallel, one batch image per core):
  - partitions = 128 = [64 out-channels of K1 | 64 out-channels of K2]
  - free dim = 900 output positions as [30 rows, 30 cols] windows (row
    stride 32) into a per-ci broadcast row; host pre-replicates the rows
    across partitions in DRAM (fp16, even+odd parity copies so every tap
    window is 4B aligned).  The per-(tap,ci) kernel scalars and the U
    clamps ride along as trailing columns of the same tile, so one DMA
    per ci delivers everything and nothing else gates the first product.
  - products on the Activation engine (Copy with per-partition scale;
    Pool rejects TensorScalarPtr/TensorTensor at codegen), grouped 4 taps
    per buffer; the seed group and a small sliver run on DVE to balance.
  - folds on DVE: per group one tensor_tensor max + one min over
    [128, 4*900] fp16 (2x_1p mode). U clamp applied once post-merge.
  - tail, pipelined in two PSUM-bank-aligned column halves: merge the
    sub-accumulators, clamp at +-U, then accumulating PE matmuls with
    lhsT=[I;-I] turn the accs into (col_K1 - col_K2) sums with channels
    on PARTITIONS ([64, *] PSUM — 64 DMA descriptor lines instead of
    900), Activation adds the bias while staging PSUM->SBUF, DMA out
    Y [64, 900] (host transposes back).
"""

import os
from contextlib import ExitStack

import numpy as np

import concourse.bass as bass
import concourse.mybir as mybir
from concourse import bacc
import concourse.tile as tile
from concourse.bass_utils import run_bass_kernel_spmd

N_CORES = 8
H = W = C = 32
COUT = 64
HO = WO = 30
NPIX = H * W          # 1024
FD = HO * WO          # 900 output positions, accessed as [30, 30] windows
ROWL = 1026           # even-parity row length (1024 pixels + 2 pad)
SCOL = 2 * ROWL       # fp32 per-tap scalars, packed as fp16 slot pairs
UCOL = SCOL + 18      # [U, -U] fp32 columns (2 slots each)
XLEN = UCOL + 4       # row length in fp16 slots (4B aligned)
P = 288               # 3*3*32 patch size
G = 4                 # taps per product buffer / fold group

F32 = mybir.dt.float32
F16 = mybir.dt.float16
_cache: dict = {}
last_results = None


def _ensure_axon_ntff_hook():
    """The trimmed agent image lacks antenv.axon_hooks; recreate it so
    run_bass_kernel_spmd(trace=True) can capture NTFF profiles. No-op on
    failure (tracing then just degrades)."""
    import sys
    import types

    try:
        import antenv.axon_hooks  # noqa: F401
        return
    except ImportError:
        pass
    try:
        mod = types.ModuleType("antenv.axon_hooks")
        holder = [None]
        mod.set_axon_ntff_profile_hook = lambda h: holder.__setitem__(0, h)
        mod.get_axon_ntff_profile_hook = lambda: holder[0]
        sys.modules["antenv.axon_hooks"] = mod
        from trn_agent_boot.trn_boot import _ntff_profile_via_ctypes

        so = "/opt/axon/libaxon_pjrt.so"
        if os.path.exists(so):
            holder[0] = _ntff_profile_via_ctypes(so)
    except Exception:
        pass


def _build_module():
    nc = bacc.Bacc()
    Alu = mybir.AluOpType

    XB = nc.dram_tensor("XB", [C * 128, XLEN], F16, kind="ExternalInput")
    M1 = nc.dram_tensor("M1", [128, COUT], F16, kind="ExternalInput")
    BCc = nc.dram_tensor("BCc", [COUT, 1], F32, kind="ExternalInput")
    Y = nc.dram_tensor("Y", [COUT, FD], F32, kind="ExternalOutput")

    with tile.TileContext(nc) as tc, ExitStack() as ctx:
        const = ctx.enter_context(tc.tile_pool(name="const", bufs=1))
        xbp = ctx.enter_context(tc.tile_pool(name="xbp", bufs=6))
        pbp = ctx.enter_context(tc.tile_pool(name="pbp", bufs=8))
        accp = ctx.enter_context(tc.tile_pool(name="accp", bufs=1))
        tps = ctx.enter_context(tc.tile_pool(name="tps", bufs=1, space="PSUM"))
        tsb = ctx.enter_context(tc.tile_pool(name="tsb", bufs=1))

        # xb[0] gates the first products: issue it before everything else.
        xb0 = xbp.tile([128, XLEN], F16, tag="xb")
        nc.sync.dma_start(out=xb0[:, :], in_=XB[0:128, :])
        M1_sb = const.tile([128, COUT], F16)
        nc.gpsimd.dma_start(out=M1_sb[:, :], in_=M1[:, :])
        BC_sb = const.tile([COUT, 1], F32)
        nc.gpsimd.dma_start(out=BC_sb[:, :], in_=BCc[:, :])

        accMax = accp.tile([128, G * FD], F16)
        accMin = accp.tile([128, G * FD], F16)

        pb = None
        xbf = xb0  # tile holding the (identical) U columns, kept live
        for ci in range(C):
            if ci == 0:
                xb_sb = xb0
            else:
                xb_sb = xbp.tile([128, XLEN], F16, tag="xb")
                nc.sync.dma_start(
                    out=xb_sb[:, :], in_=XB[ci * 128 : (ci + 1) * 128, :])
                if ci == C - 1:
                    xbf = xb_sb
            for t in range(9):
                i, j = divmod(t, 3)
                base = (ROWL + i * W) if j == 1 else (i * W + j)
                win = xb_sb[:, base : base + HO * W].rearrange(
                    "q (a b) -> q a b", b=W)[:, :, :WO]
                k = ci * 9 + t
                sc = xb_sb[:, SCOL + 2 * t : SCOL + 2 * t + 2].bitcast(F32)
                g, slot = divmod(k, G)
                if g == 0:
                    # first group seeds accMax directly; accMin is copied
                    # from it once (below) instead of duplicating products
                    dst = accMax
                elif slot == 0:
                    pb = pbp.tile([128, G * FD], F16, tag="pb")
                    dst = pb
                else:
                    dst = pb
                out_view = dst[:, slot * FD : (slot + 1) * FD].rearrange(
                    "q (a b) -> q a b", a=HO)
                if g == 0 or (k % 48 == 47 and g < 71):
                    # seed group + a sliver of products run on DVE: it is
                    # idle during ramp-up and slightly under Act's load
                    nc.vector.tensor_scalar(
                        out=out_view, in0=win, scalar1=sc, scalar2=None,
                        op0=Alu.mult)
                else:
                    nc.scalar.mul(out=out_view, in_=win, mul=sc)
                if g == 0 and slot == G - 1:
                    nc.vector.tensor_scalar(
                        out=accMin[:, :], in0=accMax[:, :], scalar1=0.0,
                        scalar2=None, op0=Alu.add)
                if 0 < g < 71 and slot == G - 1:
                    nc.vector.tensor_tensor(
                        accMax[:, :], pb[:, :], accMax[:, :], Alu.max)
                    nc.vector.tensor_tensor(
                        accMin[:, :], pb[:, :], accMin[:, :], Alu.min)

        # Tail. The accumulator merge tree (groups 0..70) runs EARLY --
        # hidden behind the Act products of the final group -- and the last
        # group's buffer is folded by a pairwise tree per column half, so
        # only ~5us of DVE work remains after the last product.
        tmpx = accp.tile([128, 2 * FD], F16)
        tmpn = accp.tile([128, 2 * FD], F16)
        M0x = accp.tile([128, FD], F16)
        M0n = accp.tile([128, FD], F16)
        nc.vector.tensor_tensor(
            tmpx[:, :], accMax[:, : 2 * FD], accMax[:, 2 * FD :], Alu.max)
        nc.vector.tensor_tensor(
            M0x[:, :], tmpx[:, :FD], tmpx[:, FD:], Alu.max)
        nc.vector.tensor_tensor(
            tmpn[:, :], accMin[:, : 2 * FD], accMin[:, 2 * FD :], Alu.min)
        nc.vector.tensor_tensor(
            M0n[:, :], tmpn[:, :FD], tmpn[:, FD:], Alu.min)

        Mx = accp.tile([128, FD], F16)
        Mn = accp.tile([128, FD], F16)
        t71x = accp.tile([128, 2 * FD], F16)
        t71n = accp.tile([128, 2 * FD], F16)
        pt = tps.tile([128, FD], F32)
        y32 = tsb.tile([COUT, FD], F32)
        pb4f = pb[:, :].rearrange("q (u f) -> q u f", f=FD)
        t71x2 = t71x[:, :].rearrange("q (u f) -> q u f", f=FD)
        t71n2 = t71n[:, :].rearrange("q (u f) -> q u f", f=FD)
        for s, e in ((0, 512), (512, FD)):
            nc.vector.tensor_tensor(
                t71x2[:, :, s:e], pb4f[:, 0:2, s:e], pb4f[:, 2:4, s:e], Alu.max)
            nc.vector.tensor_tensor(
                Mx[:, s:e], t71x[:, s:e], t71x[:, FD + s : FD + e], Alu.max)
            nc.vector.tensor_tensor(
                Mx[:, s:e], Mx[:, s:e], M0x[:, s:e], Alu.max)
            nc.vector.tensor_scalar(
                out=Mx[:, s:e], in0=Mx[:, s:e],
                scalar1=xbf[:, UCOL : UCOL + 2].bitcast(F32), scalar2=None,
                op0=Alu.max)
            nc.vector.tensor_tensor(
                t71n2[:, :, s:e], pb4f[:, 0:2, s:e], pb4f[:, 2:4, s:e], Alu.min)
            nc.vector.tensor_tensor(
                Mn[:, s:e], t71n[:, s:e], t71n[:, FD + s : FD + e], Alu.min)
            nc.vector.tensor_tensor(
                Mn[:, s:e], Mn[:, s:e], M0n[:, s:e], Alu.min)
            nc.vector.tensor_scalar(
                out=Mn[:, s:e], in0=Mn[:, s:e],
                scalar1=xbf[:, UCOL + 2 : UCOL + 4].bitcast(F32), scalar2=None,
                op0=Alu.min)
            nc.tensor.matmul(pt[:COUT, s:e], lhsT=M1_sb[:, :], rhs=Mx[:, s:e],
                             start=True, stop=False)
            nc.tensor.matmul(pt[:COUT, s:e], lhsT=M1_sb[:, :], rhs=Mn[:, s:e],
                             start=False, stop=True)
            nc.scalar.activation(
                out=y32[:, s:e], in_=pt[:COUT, s:e],
                func=mybir.ActivationFunctionType.Identity,
                bias=BC_sb[:, 0:1], scale=1.0)
            nc.sync.dma_start(out=Y[:, s:e], in_=y32[:, s:e])
    nc.finalize()
    return nc


def _host_prep(x, k1, k2, bias):
    x = np.ascontiguousarray(np.asarray(x, dtype=np.float32))
    K1 = np.exp(np.asarray(k1, np.float32).reshape(3, 3, C, COUT))
    K2 = np.exp(np.asarray(k2, np.float32).reshape(3, 3, C, COUT))
    # S[q, ci, t=i*3+j]: q<64 -> K1[i,j,ci,q];  q>=64 -> K2[i,j,ci,q-64]
    S1 = K1.transpose(3, 2, 0, 1).reshape(COUT, C, 9)
    S2 = K2.transpose(3, 2, 0, 1).reshape(COUT, C, 9)
    S = np.concatenate([S1, S2], axis=0).astype(np.float32)   # [128, C, 9]
    U1 = 0.1 * K1.reshape(9 * C, COUT).max(axis=0)
    U2_ = 0.1 * K2.reshape(9 * C, COUT).max(axis=0)
    U = np.concatenate([U1, U2_]).astype(np.float32)          # [128]
    M1 = np.vstack([np.eye(COUT, dtype=np.float16), -np.eye(COUT, dtype=np.float16)])
    BCc = np.asarray(bias, np.float32).reshape(COUT, 1)
    shared = dict(M1=np.ascontiguousarray(M1), BCc=np.ascontiguousarray(BCc))
    in_maps = []
    for n in range(N_CORES):
        rows = np.zeros((C, XLEN), np.float16)
        xr = x[n].reshape(NPIX, C).T.astype(np.float16)       # [C, 1024]
        rows[:, :NPIX] = xr
        rows[:, ROWL : ROWL + NPIX - 1] = xr[:, 1:]
        xb = np.broadcast_to(rows[:, None, :], (C, 128, XLEN)).copy()
        xb[:, :, SCOL:UCOL].view(np.float32)[:] = S.transpose(1, 0, 2)
        xb[:, :, UCOL : UCOL + 2].view(np.float32)[:, :, 0] = U[None, :]
        xb[:, :, UCOL + 2 : UCOL + 4].view(np.float32)[:, :, 0] = -U[None, :]
        in_maps.append({"XB": xb.reshape(C * 128, XLEN), **shared})
    return in_maps


def kernel(x, k1, k2, bias):
    global last_results
    if "nc" not in _cache:
        _cache["nc"] = _build_module()
    nc = _cache["nc"]
    in_maps = _host_prep(x, k1, k2, bias)
    trace = bool(int(os.environ.get("KTRACE", "0")))
    if trace:
        _ensure_axon_ntff_hook()
    res = run_bass_kernel_spmd(
        nc, in_maps, core_ids=list(range(N_CORES)), trace=trace,
    )
    last_results = res
    y = np.stack([r["Y"].reshape(COUT, HO, WO).transpose(1, 2, 0)
                  for r in res.results], axis=0)
    return np.ascontiguousarray(y, np.float32)


# revision 19
# speedup vs baseline: 1.0198x; 1.0008x over previous
"""Bipolar morphological conv2d kernel for Trainium2 (8 NeuronCores).

Math: reference computes, per output position and out-channel c,
    y = m(lp1,K1) - m(lp1,K2) - m(lp2,K1) + m(lp2,K2) + bias
with m(logp, k)[c] = exp(max_p(logp_p + k_pc)), lp1 = log(max(patch, .1)),
lp2 = log(max(-patch, .1)).

Since exp is monotone, m(lp1,K)[c] = max(U_c, max_p(x_p*K_pc)) and
m(lp2,K)[c] = max(U_c, -min_p(x_p*K_pc)) with K = exp(k) > 0 and
U_c = .1*max_p K_pc (the clamp folds into a per-channel constant).  So the
whole op needs ONE product set per kernel, max- AND min-reduced over taps:
    y = (mA1 - mA2) + (aMin1 - aMin2) + bias
with mA_k = max(U_k, max_p x_p*K_k), aMin_k = min(-U_k, min_p x_p*K_k).

Device strategy (data-parallel, one batch image per core):
  - partitions = 128 = [64 out-channels of K1 | 64 out-channels of K2]
  - free dim = 900 output positions as [30 rows, 30 cols] windows (row
    stride 32) into a per-ci broadcast row; host pre-replicates the rows
    across partitions in DRAM (fp16, even+odd parity copies so every tap
    window is 4B aligned).  The per-(tap,ci) kernel scalars and the U
    clamps ride along as trailing columns of the same tile, so one DMA
    per ci delivers everything and nothing else gates the first product.
  - products on the Activation engine (Copy with per-partition scale;
    Pool rejects TensorScalarPtr/TensorTensor at codegen), grouped 4 taps
    per buffer; the seed group and a small sliver run on DVE to balance.
  - folds on DVE: per group one tensor_tensor max + one min over
    [128, 4*900] fp16 (2x_1p mode). U clamp applied once post-merge.
  - tail, pipelined in two PSUM-bank-aligned column halves: merge the
    sub-accumulators, clamp at +-U, then accumulating PE matmuls with
    lhsT=[I;-I] turn the accs into (col_K1 - col_K2) sums with channels
    on PARTITIONS ([64, *] PSUM — 64 DMA descriptor lines instead of
    900), Activation adds the bias while staging PSUM->SBUF, DMA out
    Y [64, 900] (host transposes back).
"""

import os
from contextlib import ExitStack

import numpy as np

import concourse.bass as bass
import concourse.mybir as mybir
from concourse import bacc
import concourse.tile as tile
from concourse.bass_utils import run_bass_kernel_spmd

N_CORES = 8
H = W = C = 32
COUT = 64
HO = WO = 30
NPIX = H * W          # 1024
FD = HO * WO          # 900 output positions, accessed as [30, 30] windows
ROWL = 1026           # even-parity row length (1024 pixels + 2 pad)
SCOL = 2 * ROWL       # fp32 per-tap scalars, packed as fp16 slot pairs
UCOL = SCOL + 18      # [U, -U] fp32 columns (2 slots each)
XLEN = UCOL + 4       # row length in fp16 slots (4B aligned)
P = 288               # 3*3*32 patch size
G = 4                 # taps per product buffer / fold group

F32 = mybir.dt.float32
F16 = mybir.dt.float16
_cache: dict = {}
last_results = None


def _ensure_axon_ntff_hook():
    """The trimmed agent image lacks antenv.axon_hooks; recreate it so
    run_bass_kernel_spmd(trace=True) can capture NTFF profiles. No-op on
    failure (tracing then just degrades)."""
    import sys
    import types

    try:
        import antenv.axon_hooks  # noqa: F401
        return
    except ImportError:
        pass
    try:
        mod = types.ModuleType("antenv.axon_hooks")
        holder = [None]
        mod.set_axon_ntff_profile_hook = lambda h: holder.__setitem__(0, h)
        mod.get_axon_ntff_profile_hook = lambda: holder[0]
        sys.modules["antenv.axon_hooks"] = mod
        from trn_agent_boot.trn_boot import _ntff_profile_via_ctypes

        so = "/opt/axon/libaxon_pjrt.so"
        if os.path.exists(so):
            holder[0] = _ntff_profile_via_ctypes(so)
    except Exception:
        pass


def _build_module():
    nc = bacc.Bacc()
    Alu = mybir.AluOpType

    XB = nc.dram_tensor("XB", [C * 128, XLEN], F16, kind="ExternalInput")
    M1 = nc.dram_tensor("M1", [128, COUT], F16, kind="ExternalInput")
    BCc = nc.dram_tensor("BCc", [COUT, 1], F32, kind="ExternalInput")
    Y = nc.dram_tensor("Y", [COUT, FD], F16, kind="ExternalOutput")

    with tile.TileContext(nc) as tc, ExitStack() as ctx:
        const = ctx.enter_context(tc.tile_pool(name="const", bufs=1))
        xbp = ctx.enter_context(tc.tile_pool(name="xbp", bufs=6))
        pbp = ctx.enter_context(tc.tile_pool(name="pbp", bufs=8))
        accp = ctx.enter_context(tc.tile_pool(name="accp", bufs=1))
        tps = ctx.enter_context(tc.tile_pool(name="tps", bufs=1, space="PSUM"))
        tsb = ctx.enter_context(tc.tile_pool(name="tsb", bufs=1))

        # xb[0] gates the first products: issue it before everything else.
        xb0 = xbp.tile([128, XLEN], F16, tag="xb")
        nc.sync.dma_start(out=xb0[:, :], in_=XB[0:128, :])
        M1_sb = const.tile([128, COUT], F16)
        nc.gpsimd.dma_start(out=M1_sb[:, :], in_=M1[:, :])
        BC_sb = const.tile([COUT, 1], F32)
        nc.gpsimd.dma_start(out=BC_sb[:, :], in_=BCc[:, :])

        accMax = accp.tile([128, G * FD], F16)
        accMin = accp.tile([128, G * FD], F16)

        pb = None
        xbf = xb0  # tile holding the (identical) U columns, kept live
        for ci in range(C):
            if ci == 0:
                xb_sb = xb0
            else:
                xb_sb = xbp.tile([128, XLEN], F16, tag="xb")
                nc.sync.dma_start(
                    out=xb_sb[:, :], in_=XB[ci * 128 : (ci + 1) * 128, :])
                if ci == C - 1:
                    xbf = xb_sb
            for t in range(9):
                i, j = divmod(t, 3)
                base = (ROWL + i * W) if j == 1 else (i * W + j)
                win = xb_sb[:, base : base + HO * W].rearrange(
                    "q (a b) -> q a b", b=W)[:, :, :WO]
                k = ci * 9 + t
                sc = xb_sb[:, SCOL + 2 * t : SCOL + 2 * t + 2].bitcast(F32)
                g, slot = divmod(k, G)
                if g == 0:
                    # first group seeds accMax directly; accMin is copied
                    # from it once (below) instead of duplicating products
                    dst = accMax
                elif slot == 0:
                    pb = pbp.tile([128, G * FD], F16, tag="pb")
                    dst = pb
                else:
                    dst = pb
                out_view = dst[:, slot * FD : (slot + 1) * FD].rearrange(
                    "q (a b) -> q a b", a=HO)
                if g == 0 or (k % 48 == 47 and g < 71):
                    # seed group + a sliver of products run on DVE: it is
                    # idle during ramp-up and slightly under Act's load
                    nc.vector.tensor_scalar(
                        out=out_view, in0=win, scalar1=sc, scalar2=None,
                        op0=Alu.mult)
                else:
                    nc.scalar.mul(out=out_view, in_=win, mul=sc)
                if g == 0 and slot == G - 1:
                    nc.vector.tensor_scalar(
                        out=accMin[:, :], in0=accMax[:, :], scalar1=0.0,
                        scalar2=None, op0=Alu.add)
                if 0 < g < 71 and slot == G - 1:
                    nc.vector.tensor_tensor(
                        accMax[:, :], pb[:, :], accMax[:, :], Alu.max)
                    nc.vector.tensor_tensor(
                        accMin[:, :], pb[:, :], accMin[:, :], Alu.min)

        # Tail. The accumulator merge tree (groups 0..70) runs EARLY --
        # hidden behind the Act products of the final group -- and the last
        # group's buffer is folded by a pairwise tree per column half, so
        # only ~5us of DVE work remains after the last product.
        tmpx = accp.tile([128, 2 * FD], F16)
        tmpn = accp.tile([128, 2 * FD], F16)
        M0x = accp.tile([128, FD], F16)
        M0n = accp.tile([128, FD], F16)
        nc.vector.tensor_tensor(
            tmpx[:, :], accMax[:, : 2 * FD], accMax[:, 2 * FD :], Alu.max)
        nc.vector.tensor_tensor(
            M0x[:, :], tmpx[:, :FD], tmpx[:, FD:], Alu.max)
        nc.vector.tensor_tensor(
            tmpn[:, :], accMin[:, : 2 * FD], accMin[:, 2 * FD :], Alu.min)
        nc.vector.tensor_tensor(
            M0n[:, :], tmpn[:, :FD], tmpn[:, FD:], Alu.min)

        Mx = accp.tile([128, FD], F16)
        Mn = accp.tile([128, FD], F16)
        t71x = accp.tile([128, 2 * FD], F16)
        t71n = accp.tile([128, 2 * FD], F16)
        pt = tps.tile([128, FD], F32)
        y32 = tsb.tile([COUT, FD], F16)
        pb4f = pb[:, :].rearrange("q (u f) -> q u f", f=FD)
        t71x2 = t71x[:, :].rearrange("q (u f) -> q u f", f=FD)
        t71n2 = t71n[:, :].rearrange("q (u f) -> q u f", f=FD)
        for s, e in ((0, 512), (512, FD)):
            nc.vector.tensor_tensor(
                t71x2[:, :, s:e], pb4f[:, 0:2, s:e], pb4f[:, 2:4, s:e], Alu.max)
            nc.vector.tensor_tensor(
                Mx[:, s:e], t71x[:, s:e], t71x[:, FD + s : FD + e], Alu.max)
            nc.vector.tensor_tensor(
                Mx[:, s:e], Mx[:, s:e], M0x[:, s:e], Alu.max)
            nc.vector.tensor_scalar(
                out=Mx[:, s:e], in0=Mx[:, s:e],
                scalar1=xbf[:, UCOL : UCOL + 2].bitcast(F32), scalar2=None,
                op0=Alu.max)
            nc.vector.tensor_tensor(
                t71n2[:, :, s:e], pb4f[:, 0:2, s:e], pb4f[:, 2:4, s:e], Alu.min)
            nc.vector.tensor_tensor(
                Mn[:, s:e], t71n[:, s:e], t71n[:, FD + s : FD + e], Alu.min)
            nc.vector.tensor_tensor(
                Mn[:, s:e], Mn[:, s:e], M0n[:, s:e], Alu.min)
            nc.vector.tensor_scalar(
                out=Mn[:, s:e], in0=Mn[:, s:e],
                scalar1=xbf[:, UCOL + 2 : UCOL + 4].bitcast(F32), scalar2=None,
                op0=Alu.min)
            nc.tensor.matmul(pt[:COUT, s:e], lhsT=M1_sb[:, :], rhs=Mx[:, s:e],
                             start=True, stop=False)
            nc.tensor.matmul(pt[:COUT, s:e], lhsT=M1_sb[:, :], rhs=Mn[:, s:e],
                             start=False, stop=True)
            nc.scalar.activation(
                out=y32[:, s:e], in_=pt[:COUT, s:e],
                func=mybir.ActivationFunctionType.Identity,
                bias=BC_sb[:, 0:1], scale=1.0)
            if s == 0:
                nc.sync.dma_start(out=Y[:, s:e], in_=y32[:, s:e])
            else:
                nc.gpsimd.dma_start(out=Y[:, s:e], in_=y32[:, s:e])
    nc.finalize()
    return nc


def _host_prep(x, k1, k2, bias):
    x = np.ascontiguousarray(np.asarray(x, dtype=np.float32))
    K1 = np.exp(np.asarray(k1, np.float32).reshape(3, 3, C, COUT))
    K2 = np.exp(np.asarray(k2, np.float32).reshape(3, 3, C, COUT))
    # S[q, ci, t=i*3+j]: q<64 -> K1[i,j,ci,q];  q>=64 -> K2[i,j,ci,q-64]
    S1 = K1.transpose(3, 2, 0, 1).reshape(COUT, C, 9)
    S2 = K2.transpose(3, 2, 0, 1).reshape(COUT, C, 9)
    S = np.concatenate([S1, S2], axis=0).astype(np.float32)   # [128, C, 9]
    U1 = 0.1 * K1.reshape(9 * C, COUT).max(axis=0)
    U2_ = 0.1 * K2.reshape(9 * C, COUT).max(axis=0)
    U = np.concatenate([U1, U2_]).astype(np.float32)          # [128]
    M1 = np.vstack([np.eye(COUT, dtype=np.float16), -np.eye(COUT, dtype=np.float16)])
    BCc = np.asarray(bias, np.float32).reshape(COUT, 1)
    shared = dict(M1=np.ascontiguousarray(M1), BCc=np.ascontiguousarray(BCc))
    in_maps = []
    for n in range(N_CORES):
        rows = np.zeros((C, XLEN), np.float16)
        xr = x[n].reshape(NPIX, C).T.astype(np.float16)       # [C, 1024]
        rows[:, :NPIX] = xr
        rows[:, ROWL : ROWL + NPIX - 1] = xr[:, 1:]
        xb = np.broadcast_to(rows[:, None, :], (C, 128, XLEN)).copy()
        xb[:, :, SCOL:UCOL].view(np.float32)[:] = S.transpose(1, 0, 2)
        xb[:, :, UCOL : UCOL + 2].view(np.float32)[:, :, 0] = U[None, :]
        xb[:, :, UCOL + 2 : UCOL + 4].view(np.float32)[:, :, 0] = -U[None, :]
        in_maps.append({"XB": xb.reshape(C * 128, XLEN), **shared})
    return in_maps


def kernel(x, k1, k2, bias):
    global last_results
    if "nc" not in _cache:
        _cache["nc"] = _build_module()
    nc = _cache["nc"]
    in_maps = _host_prep(x, k1, k2, bias)
    trace = bool(int(os.environ.get("KTRACE", "0")))
    if trace:
        _ensure_axon_ntff_hook()
    res = run_bass_kernel_spmd(
        nc, in_maps, core_ids=list(range(N_CORES)), trace=trace,
    )
    last_results = res
    y = np.stack([r["Y"].astype(np.float32).reshape(COUT, HO, WO).transpose(1, 2, 0)
                  for r in res.results], axis=0)
    return np.ascontiguousarray(y, np.float32)
